# revision 3
# baseline (speedup 1.0000x reference)
"""GCN (3-layer, PyG GCNConv-style) + mean-pool + MLP head on 8 Trainium2 NeuronCores.

v3: v2 plus
 - 5 buckets: bucket 0 = src on MY core (gathered straight from the local z
   buffer, no AllGather dependency), buckets 1-4 = remote srcs by quarter
   table. Remote caps shrink 640 -> 512 (-15% descriptors / P bytes / matmuls).
 - Dual z DRAM buffers (z2 / z3) so local-bucket gathers of layer L never race
   layer L+1's z writes.
 - Pooling one-hot (graph-id) matrix precomputed on host, resident in SBUF.
 - Zero-bias fast path: hc = relu((acc + z_self) * dinv) via one DVE add + one
   ACT scale+relu (vs two scalar_tensor_tensor).
 - Layer-1 matmuls back in lhsT=edge-values orientation (1-column LDWEIGHTS),
   with the rank-2 b1==0 shortcut for z2.
 - Wave-skewed gather issue over 6 staging buffers to keep SWDGE busy while
   the (serialized, ~80us each) quarter AllGathers complete.
"""
import sys
import numpy as np

sys.path.insert(0, "/opt/trn_rl_repo")

NCORES = 8
P = 128
NQ = 4          # quarter tables
NBUCKT = 4      # 4 quarter tables


def _roundup(x, m):
    return (x + m - 1) // m * m


def _wrap_idxs(v):
    L = v.shape[0]
    return np.tile(v.reshape(L // 16, 16).T, (8, 1))


def _slotmajor(v):
    L = v.shape[0]
    return np.ascontiguousarray(v.reshape(L // 128, 128).T)


def preprocess(x, edge_index, batch, svm_pred, G=2):
    import ml_dtypes
    bf16 = ml_dtypes.bfloat16

    N = x.shape[0]
    E = edge_index.shape[1]
    B = svm_pred.shape[0]
    H = 256
    gpc = B // NCORES

    x = np.asarray(x, np.float32)
    ei = np.asarray(edge_index, np.int64)
    batch = np.asarray(batch, np.int64)

    node_start = np.searchsorted(batch, np.arange(NCORES) * gpc)
    node_end = np.searchsorted(batch, np.arange(1, NCORES + 1) * gpc)
    cnts = node_end - node_start
    NC = int(_roundup(_roundup(cnts.max(), P) // P, G))
    if NC % NQ:
        NC = int(_roundup(NC, G * NQ))
    NP = NC * P
    QS = NP // NQ
    RQ = NCORES * QS
    assert RQ <= 32768 and NP <= 32768
    assert NC % NQ == 0 and NC % G == 0

    src, dst = ei[0], ei[1]
    deg = (np.bincount(dst, minlength=N) + 1.0).astype(np.float32)
    dinv = deg ** -0.5
    norm = dinv[src] * dinv[dst]

    core_of = (batch // gpc).astype(np.int64)
    local_of = np.arange(N) - node_start[core_of]

    xg = x[:, 0][src] * norm
    xself = x[:, 0] * dinv * dinv

    # bucket of each edge depends on consumer core: local vs quarter of src
    percore = []
    maxq = 0
    for c in range(NCORES):
        m = np.where(core_of[dst] == c)[0]
        ldst = (dst[m] - node_start[c]).astype(np.int64)
        sc = core_of[src[m]]
        sl = local_of[src[m]]
        nb = sl // QS
        srel = (sc * QS + (sl % QS)).astype(np.int16)
        chunk = ldst >> 7
        slot = (ldst & 127).astype(np.float32)
        key = chunk * NBUCKT + nb
        order = np.argsort(key, kind="stable")
        bounds = np.searchsorted(key[order], np.arange(NC * NBUCKT + 1))
        gc = np.diff(bounds).reshape(NC, NBUCKT)
        maxq = max(maxq, int(gc.max()))
        percore.append((order, bounds, srel, slot, xg[m].astype(np.float32)))

    CAPQ = int(_roundup(max(maxq, 128), P))
    CAPL = CAPQ
    caps = [CAPQ] * NQ
    off = np.concatenate([[0], np.cumsum(caps)])  # per-bucket slot offset
    SLOTC = int(off[-1])                          # slots per chunk
    TT = SLOTC // P                               # tiles per chunk
    tb = [c // P for c in caps]                   # tiles per bucket
    tilesum = np.concatenate([[0], np.cumsum(tb)])
    NG = NC // G
    NSLOT = NC * SLOTC

    # call-major idx layout: call (g, b) covers G chunks' b-segments
    callw = [G * c // 16 for c in caps]           # idx cols per call
    callcol = np.zeros(NG * NBUCKT + 1, np.int64)
    for call in range(NG * NBUCKT):
        callcol[call + 1] = callcol[call] + callw[call % NBUCKT]
    IDXCOLS = int(callcol[-1])

    gcnt = np.bincount(batch, minlength=B).astype(np.float32)
    invc = 1.0 / np.maximum(gcnt, 1.0)

    in_maps = []
    for c in range(NCORES):
        order, bounds, srel, slot, xgv = percore[c]
        slotf = np.full(NSLOT, -1.0, np.float32)
        xgf = np.zeros(NSLOT, np.float32)
        idxw = np.zeros((P, IDXCOLS), np.int16)
        for ch in range(NC):
            g, j = ch // G, ch % G
            for b in range(NBUCKT):
                k = ch * NBUCKT + b
                lo, hi = bounds[k], bounds[k + 1]
                n = hi - lo
                sel = order[lo:hi]
                mbase = ch * SLOTC + off[b]
                slotf[mbase:mbase + n] = slot[sel]
                xgf[mbase:mbase + n] = xgv[sel]
                call = g * NBUCKT + b
                seg = np.zeros(caps[b], np.int16)
                seg[:n] = srel[sel]
                cw = caps[b] // 16
                c0 = callcol[call] + j * cw
                idxw[:, c0:c0 + cw] = _wrap_idxs(seg)

        n = cnts[c]
        gidl = np.full(NP, -1.0, np.float32)
        gidl[:n] = batch[node_start[c]:node_end[c]].astype(np.float32)
        dvl = np.zeros(NP, np.float32)
        dvl[:n] = dinv[node_start[c]:node_end[c]]
        xsl = np.zeros(NP, np.float32)
        xsl[:n] = xself[node_start[c]:node_end[c]]

        slotm = _slotmajor(slotf)
        p01 = (slotm[:, :, None] == np.arange(P, dtype=np.float32)).astype(
            ml_dtypes.float8_e4m3)
        ohall = (gidl.reshape(NC, P).T[:, :, None] ==
                 np.arange(B, dtype=np.float32)).astype(np.float32)  # [P,NC,B]
        in_maps.append({
            "idxw": idxw,
            "p01": np.ascontiguousarray(p01.reshape(P, (NSLOT // P) * P)),
            "xgv": _slotmajor(xgf).astype(bf16),
            "ohall": np.ascontiguousarray(ohall.reshape(P, NC * B)),
            "dinvl": np.ascontiguousarray(dvl.reshape(NC, P).T),
            "xsf": np.ascontiguousarray(xsl.reshape(NC, P).T),
            "xsfn": np.ascontiguousarray((-xsl).reshape(NC, P).T),
        })

    params = dict(N=N, E=E, B=B, H=H, NP=NP, NC=NC, QS=QS, RQ=RQ,
                  CAPL=CAPL, CAPQ=CAPQ, G=G, NG=NG, NSLOT=NSLOT, TT=TT,
                  SLOTC=SLOTC, IDXCOLS=IDXCOLS,
                  caps=caps, tb=tb, tilesum=[int(t) for t in tilesum],
                  callcol=[int(t) for t in callcol])
    return params, in_maps, invc


def add_weight_inputs(in_maps, params, W1, b1, W2, b2, W3, b3, Wf1, bf1, Wf2, bf2,
                      svm_pred, invc):
    import ml_dtypes
    bf16 = ml_dtypes.bfloat16
    B, H = params["B"], params["H"]
    f32 = np.float32

    def kswiz(W, width):
        W = np.asarray(W, f32)
        return np.ascontiguousarray(
            W.reshape(2, P, width).transpose(1, 0, 2).reshape(P, 2 * width))

    W1r = np.asarray(W1, f32).reshape(H)
    u = np.maximum(W1r, 0.0) @ np.asarray(W2, f32)
    v = np.maximum(-W1r, 0.0) @ np.asarray(W2, f32)

    shared = {
        "urep": np.repeat(u.reshape(1, H), P, 0),
        "vrep": np.repeat(v.reshape(1, H), P, 0),
        "W1rep": np.repeat(np.asarray(W1, f32).reshape(1, H), P, 0),
        "b1rep": np.repeat(np.asarray(b1, f32).reshape(1, H), P, 0),
        "W2s": kswiz(W2, H).astype(bf16),
        "W3s": kswiz(W3, H).astype(bf16),
        "b2rep": np.repeat(np.asarray(b2, f32).reshape(1, H), P, 0),
        "b3rep": np.repeat(np.asarray(b3, f32).reshape(1, H), P, 0),
        "Wf1k": kswiz(np.asarray(Wf1, f32)[:2 * P], 128),
        "Wf1c": np.ascontiguousarray(np.asarray(Wf1, f32)[2 * P:].reshape(1, 128)),
        "bf1rep": np.repeat(np.asarray(bf1, f32).reshape(1, 128), B, 0),
        "Wf2s": np.asarray(Wf2, f32).reshape(P, 6),
        "bf2rep": np.repeat(np.asarray(bf2, f32).reshape(1, 6), B, 0),
        "svm": np.asarray(svm_pred, f32).reshape(1, B),
        "invc2": np.tile(np.asarray(invc, f32).reshape(1, 1, B),
                         (P, 2, 1)).reshape(P, 2 * B),
    }
    for m in in_maps:
        m.update(shared)


def build(params, cut=0):
    import concourse.bacc as bacc
    import concourse.tile as tile
    from concourse import mybir
    from concourse.masks import make_identity

    NP, NC, QS, RQ = params["NP"], params["NC"], params["QS"], params["RQ"]
    G, NG = params["G"], params["NG"]
    NSLOT, TT, B, H = params["NSLOT"], params["TT"], params["B"], params["H"]
    SLOTC, IDXCOLS = params["SLOTC"], params["IDXCOLS"]
    caps, tb = params["caps"], params["tb"]
    tilesum, callcol = params["tilesum"], params["callcol"]
    QC = NC // NQ
    GT = G * TT              # msgs staging tiles per group
    NBUF = 6
    l1_fast = bool(params.get("l1_fast", True))
    b2z = bool(params.get("b2z", False))
    b3z = bool(params.get("b3z", False))

    FT = mybir.dt.float32
    BF = mybir.dt.bfloat16
    F8 = mybir.dt.float8e4
    I16 = mybir.dt.int16
    AL = mybir.AluOpType
    AF = mybir.ActivationFunctionType

    nc = bacc.Bacc("TRN2", target_bir_lowering=False, debug=False,
                   num_devices=NCORES, num_swdge_queues=4)

    dp = nc.declare_dram_parameter
    pr = {
        "idxw": dp("idxw", [P, IDXCOLS], I16, isOutput=False),
        "p01": dp("p01", [P, (NSLOT // P) * P], F8, isOutput=False),
        "xgv": dp("xgv", [P, NSLOT // P], BF, isOutput=False),
        "ohall": dp("ohall", [P, NC * B], FT, isOutput=False),
        "dinvl": dp("dinvl", [P, NC], FT, isOutput=False),
        "xsf": dp("xsf", [P, NC], FT, isOutput=False),
        "xsfn": dp("xsfn", [P, NC], FT, isOutput=False),
        "urep": dp("urep", [P, H], FT, isOutput=False),
        "vrep": dp("vrep", [P, H], FT, isOutput=False),
        "W3s": dp("W3s", [P, 2 * H], BF, isOutput=False),
        "b2rep": dp("b2rep", [P, H], FT, isOutput=False),
        "b3rep": dp("b3rep", [P, H], FT, isOutput=False),
        "Wf1k": dp("Wf1k", [P, 2 * 128], FT, isOutput=False),
        "Wf1c": dp("Wf1c", [1, 128], FT, isOutput=False),
        "bf1rep": dp("bf1rep", [B, 128], FT, isOutput=False),
        "Wf2s": dp("Wf2s", [P, 6], FT, isOutput=False),
        "bf2rep": dp("bf2rep", [B, 6], FT, isOutput=False),
        "svm": dp("svm", [1, B], FT, isOutput=False),
        "invc2": dp("invc2", [P, 2 * B], FT, isOutput=False),
    }
    if not l1_fast:
        pr["W1rep"] = dp("W1rep", [P, H], FT, isOutput=False)
        pr["b1rep"] = dp("b1rep", [P, H], FT, isOutput=False)
        pr["W2s"] = dp("W2s", [P, 2 * H], BF, isOutput=False)
    out_p = dp("out", [B, 6], FT, isOutput=True)

    with tile.TileContext(nc) as tc:
        with (
            tc.tile_pool(name="res", bufs=1) as res,
            tc.tile_pool(name="work", bufs=3) as work,
            tc.tile_pool(name="pp_acc", bufs=2, space="PSUM") as pp_acc,
            tc.tile_pool(name="pp_z", bufs=2, space="PSUM") as pp_z,
            tc.tile_pool(name="pp_t", bufs=2, space="PSUM") as pp_t,
            tc.tile_pool(name="pp_pool", bufs=1, space="PSUM") as pp_pool,
            tc.tile_pool(name="dram", bufs=1, space="DRAM") as dram,
        ):
            zloc2 = dram.tile([NP, H], F8, name="zloc2")
            zloc3 = dram.tile([NP, H], F8, name="zloc3")
            tabs2 = [dram.tile([RQ, H], F8, addr_space="Shared", name=f"t2q{q}")
                     for q in range(NQ)]
            tabs3 = [dram.tile([RQ, H], F8, addr_space="Shared", name=f"t3q{q}")
                     for q in range(NQ)]
            ccin = dram.tile([P, 2 * B], FT, name="ccin")
            ccout = dram.tile([P, 2 * B], FT, addr_space="Shared", name="ccout")

            sizes = {
                "idxw": ([P, IDXCOLS], I16),
                "xgv": ([P, NSLOT // P], BF),
                "ohall": ([P, NC * B], FT),
                "dinvl": ([P, NC], FT),
                "xsf": ([P, NC], FT),
                "xsfn": ([P, NC], FT),
                "urep": ([P, H], FT),
                "vrep": ([P, H], FT),
                "W3s": ([P, 2 * H], BF),
                "b2rep": ([P, H], FT),
                "b3rep": ([P, H], FT),
                "Wf1k": ([P, 2 * 128], FT),
                "Wf1c": ([1, 128], FT),
                "bf1rep": ([B, 128], FT),
                "Wf2s": ([P, 6], FT),
                "bf2rep": ([B, 6], FT),
                "svm": ([1, B], FT),
                "invc2": ([P, 2 * B], FT),
            }
            if not l1_fast:
                sizes["W1rep"] = ([P, H], FT)
                sizes["b1rep"] = ([P, H], FT)
                sizes["W2s"] = ([P, 2 * H], BF)
            sb = {}
            for k, (shape, dt) in sizes.items():
                sb[k] = res.tile(shape, dt, name=f"sb_{k}")
                nc.sync.dma_start(sb[k][:], pr[k][:])
            p01_dram = pr["p01"]

            ident = res.tile([P, P], FT)
            make_identity(nc, ident[:])

            zsb = res.tile([P, NC, H], F8, name="zsb")

            msgs = [res.tile([P, GT, H], F8, name=f"msgs{i}") for i in range(NBUF)]
            for _m in msgs:
                nc.vector.memset(_m[:], 0.0)

            def build_P(ch):
                Pt = work.tile([P, TT * P], F8, tag="P")
                lo = ch * TT * P
                nc.sync.dma_start(Pt[:], p01_dram[:, lo:lo + TT * P])
                return Pt

            def h_to_z(hc, W_sb, ch, zdst):
                hT = work.tile([P, 2, P], BF, tag="hT")
                for k in range(2):
                    tp = pp_t.tile([P, P], FT, tag="tp", space="PSUM")
                    nc.tensor.transpose(out=tp[:], in_=hc[:, k * P:(k + 1) * P],
                                        identity=ident[:])
                    nc.vector.tensor_copy(hT[:, k, :], tp[:])
                zp = pp_z.tile([P, H], FT, tag="zp", space="PSUM")
                for k in range(2):
                    nc.tensor.matmul(zp[:], lhsT=hT[:, k, :],
                                     rhs=W_sb[:, k * H:(k + 1) * H],
                                     start=(k == 0), stop=(k == 1))
                nc.vector.tensor_scalar(out=zsb[:, ch, :], in0=zp[:],
                                        scalar1=sb["dinvl"][:, ch:ch + 1],
                                        scalar2=None, op0=AL.mult)
                nc.sync.dma_start(zdst[ch * P:(ch + 1) * P, :], zsb[:, ch, :])

            def emit_ag(q, zsrc, tabs):
                nc.gpsimd.collective_compute(
                    "AllGather", AL.bypass,
                    replica_groups=[list(range(NCORES))],
                    ins=[zsrc[q * QS:(q + 1) * QS, :]], outs=[tabs[q].opt()])

            # ================= LAYER 1 =================
            for ch in range(NC):
                Pt = build_P(ch)
                sT = pp_acc.tile([1, P], FT, tag="acc", space="PSUM")
                for t in range(TT):
                    col = ch * TT + t
                    nc.tensor.matmul(sT[:], lhsT=sb["xgv"][:, col:col + 1],
                                     rhs=Pt[:, t * P:(t + 1) * P],
                                     start=(t == 0), stop=(t == TT - 1))
                sTr = work.tile([1, P], FT, tag="sTr")
                nc.vector.tensor_copy(sTr[:], sT[:])
                sP = pp_z.tile([P, 1], FT, tag="zp", space="PSUM")
                nc.tensor.transpose(out=sP[:, 0:1], in_=sTr[:],
                                    identity=ident[0:1, 0:1])
                if l1_fast:
                    tpos = work.tile([P, 1], FT, tag="tp1")
                    nc.scalar.activation(out=tpos[:], in_=sP[:, 0:1], func=AF.Relu,
                                         bias=sb["xsf"][:, ch:ch + 1], scale=1.0)
                    tneg = work.tile([P, 1], FT, tag="tn1")
                    nc.scalar.activation(out=tneg[:], in_=sP[:, 0:1], func=AF.Relu,
                                         bias=sb["xsfn"][:, ch:ch + 1], scale=-1.0)
                    acol = work.tile([P, 1], FT, tag="ac1")
                    nc.vector.tensor_tensor(out=acol[:], in0=tpos[:],
                                            in1=sb["dinvl"][:, ch:ch + 1],
                                            op=AL.mult)
                    bcol = work.tile([P, 1], FT, tag="bc1")
                    nc.vector.tensor_tensor(out=bcol[:], in0=tneg[:],
                                            in1=sb["dinvl"][:, ch:ch + 1],
                                            op=AL.mult)
                    tmp = work.tile([P, H], FT, tag="hc")
                    nc.vector.tensor_scalar(out=tmp[:], in0=sb["vrep"][:],
                                            scalar1=bcol[:], scalar2=None,
                                            op0=AL.mult)
                    nc.vector.scalar_tensor_tensor(
                        out=zsb[:, ch, :], in0=sb["urep"][:], scalar=acol[:],
                        in1=tmp[:], op0=AL.mult, op1=AL.add)
                    nc.sync.dma_start(zloc2[ch * P:(ch + 1) * P, :], zsb[:, ch, :])
                else:
                    scol = work.tile([P, 1], FT, tag="ac1")
                    nc.vector.tensor_tensor(out=scol[:], in0=sP[:, 0:1],
                                            in1=sb["xsf"][:, ch:ch + 1], op=AL.add)
                    h1 = work.tile([P, H], FT, tag="hc")
                    nc.vector.scalar_tensor_tensor(
                        out=h1[:], in0=sb["W1rep"][:], scalar=scol[:],
                        in1=sb["b1rep"][:], op0=AL.mult, op1=AL.add)
                    nc.scalar.activation(out=h1[:], in_=h1[:], func=AF.Relu)
                    h_to_z(h1, sb["W2s"], ch, zloc2)
                if (ch + 1) % QC == 0:
                    emit_ag(ch // QC, zloc2, tabs2)

            if cut == 1:
                fin0 = work.tile([B, 6], FT, tag="fin")
                nc.vector.memset(fin0[:], 0.0)
                nc.sync.dma_start(out_p[:], fin0[:])

            # ================= LAYERS 2,3 =================
            poolTs = [pp_pool.tile([P, B], FT, tag=f"pool{k}", space="PSUM",
                                   name=f"poolT{k}") for k in range(2)]

            def issue_gather(g, b, zcur, tabs):
                mt = msgs[g % NBUF]
                call = g * NBUCKT + b
                cw = G * caps[b] // 16
                tab = tabs[b][:]
                nc.gpsimd.dma_gather(
                    mt[:, G * tilesum[b]:G * tilesum[b + 1], :],
                    tab,
                    sb["idxw"][:, callcol[call]:callcol[call] + cw],
                    G * caps[b], G * caps[b], H, single_packet=False,
                    queue_num=b % 4)
                return mt

            def msg_layer(zcur, tabs, brow, bz, is_last, sub=4, next_z=None,
                          next_tabs=None):
                # prime the pipeline: stage the first NBUF groups
                for g in range(min(NBUF, NG)):
                    for b in range(NBUCKT):
                        issue_gather(g, b, zcur, tabs)
                for g in range(NG):
                    mt = msgs[g % NBUF]
                    if sub >= 2:
                        for j in range(G):
                            ch = g * G + j
                            Pt = build_P(ch)
                            acc = pp_acc.tile([P, H], FT, tag="acc", space="PSUM")
                            i = 0
                            for b in range(NBUCKT):
                                for t in range(tb[b]):
                                    nc.tensor.matmul(
                                        acc[:], lhsT=Pt[:, i * P:(i + 1) * P],
                                        rhs=mt[:, (tilesum[b] * G + j * tb[b]
                                                   + t), :],
                                        start=(i == 0),
                                        stop=(i == TT - 1))
                                    i += 1
                            if sub == 2:
                                continue
                            hc = work.tile([P, H], FT, tag="hc")
                            if bz:
                                nc.vector.tensor_tensor(
                                    out=hc[:], in0=acc[:], in1=zsb[:, ch, :],
                                    op=AL.add)
                                nc.scalar.activation(
                                    out=hc[:], in_=hc[:], func=AF.Relu,
                                    scale=sb["dinvl"][:, ch:ch + 1])
                            else:
                                nc.vector.scalar_tensor_tensor(
                                    out=hc[:], in0=acc[:],
                                    scalar=sb["dinvl"][:, ch:ch + 1],
                                    in1=brow[:], op0=AL.mult, op1=AL.add)
                                nc.vector.scalar_tensor_tensor(
                                    out=hc[:], in0=zsb[:, ch, :],
                                    scalar=sb["dinvl"][:, ch:ch + 1],
                                    in1=hc[:], op0=AL.mult, op1=AL.add)
                                nc.scalar.activation(out=hc[:], in_=hc[:],
                                                     func=AF.Relu)
                            if sub == 3:
                                continue
                            if not is_last:
                                h_to_z(hc, sb["W3s"], ch, next_z)
                                if (ch + 1) % QC == 0:
                                    emit_ag(ch // QC, next_z, next_tabs)
                            else:
                                for k in range(2):
                                    nc.tensor.matmul(
                                        poolTs[k][:],
                                        lhsT=hc[:, k * P:(k + 1) * P],
                                        rhs=sb["ohall"][:, ch * B:(ch + 1) * B],
                                        start=(ch == 0), stop=(ch == NC - 1))
                    # refill: issue all buckets for group g+NBUF
                    gn = g + NBUF
                    if gn < NG:
                        for b in range(NBUCKT):
                            issue_gather(gn, b, zcur, tabs)

            if cut != 1:
                msg_layer(zloc2, tabs2, sb["b2rep"], b2z, False,
                          sub=(cut - 20 if 20 < cut < 25 else 4),
                          next_z=zloc3, next_tabs=tabs3)
            if cut == 2 or 20 < cut < 25:
                fin0 = work.tile([B, 6], FT, tag="fin")
                nc.vector.memset(fin0[:], 0.0)
                nc.sync.dma_start(out_p[:], fin0[:])
            if cut == 0:
                msg_layer(zloc3, tabs3, sb["b3rep"], b3z, True)

            if cut == 0:
                poolsb = work.tile([P, 2 * B], FT, tag="poolsb")
                for k in range(2):
                    nc.vector.tensor_copy(poolsb[:, k * B:(k + 1) * B], poolTs[k][:])
                nc.sync.dma_start(ccin[:], poolsb[:])
                nc.gpsimd.collective_compute(
                    "AllReduce", AL.add, replica_groups=[list(range(NCORES))],
                    ins=[ccin.opt()], outs=[ccout.opt()])
                pooledT = work.tile([P, 2 * B], FT, tag="pooledT")
                nc.sync.dma_start(pooledT[:], ccout[:])
                nc.vector.tensor_tensor(out=pooledT[:], in0=pooledT[:],
                                        in1=sb["invc2"][:], op=AL.mult)

                o1 = pp_acc.tile([B, 128], FT, tag="acc", space="PSUM")
                pT = pooledT[:].rearrange("p (k b) -> p k b", k=2)
                for k in range(2):
                    nc.tensor.matmul(o1[:], lhsT=pT[:, k, :],
                                     rhs=sb["Wf1k"][:, k * 128:(k + 1) * 128],
                                     start=(k == 0), stop=False)
                nc.tensor.matmul(o1[:], lhsT=sb["svm"][:], rhs=sb["Wf1c"][:],
                                 start=False, stop=True)
                a1 = work.tile([B, 128], FT, tag="a1")
                nc.vector.scalar_tensor_tensor(out=a1[:], in0=o1[:], scalar=1.0,
                                               in1=sb["bf1rep"][:], op0=AL.mult,
                                               op1=AL.add)
                nc.scalar.activation(out=a1[:], in_=a1[:], func=AF.Relu)
                tpa = pp_t.tile([P, B], FT, tag="tp", space="PSUM")
                nc.tensor.transpose(out=tpa[:], in_=a1[:], identity=ident[0:B, 0:B])
                a1T = work.tile([P, B], FT, tag="a1T")
                nc.vector.tensor_copy(a1T[:], tpa[:])
                o2 = pp_z.tile([B, 6], FT, tag="zp", space="PSUM")
                nc.tensor.matmul(o2[:], lhsT=a1T[:], rhs=sb["Wf2s"][:],
                                 start=True, stop=True)
                fin = work.tile([B, 6], FT, tag="fin")
                nc.vector.scalar_tensor_tensor(out=fin[:], in0=o2[:], scalar=1.0,
                                               in1=sb["bf2rep"][:], op0=AL.mult,
                                               op1=AL.add)
                nc.sync.dma_start(out_p[:], fin[:])

    nc.compile()
    return nc


def kernel(x, edge_index, batch, svm_pred,
           W1, b1, W2, b2, W3, b3, Wf1, bf1, Wf2, bf2, **kw):
    from concourse.bass_utils import run_bass_kernel_spmd
    params, in_maps, invc = preprocess(x, edge_index, batch, svm_pred)
    add_weight_inputs(in_maps, params, W1, b1, W2, b2, W3, b3, Wf1, bf1, Wf2, bf2,
                      svm_pred, invc)
    params["l1_fast"] = not np.any(np.asarray(b1))
    params["b2z"] = not np.any(np.asarray(b2))
    params["b3z"] = not np.any(np.asarray(b3))
    if params["l1_fast"]:
        for m in in_maps:
            m.pop("W1rep", None); m.pop("b1rep", None); m.pop("W2s", None)
    nc = build(params)
    res = run_bass_kernel_spmd(nc, in_maps, core_ids=list(range(NCORES)), **kw)
    out = np.asarray(res.results[0]["out"], np.float32)
    if kw:
        return out, res
    return out
